# revision 24
# baseline (speedup 1.0000x reference)
"""NonLocalBlock (single-head attention, N=HW=4096, d=128) on 8 trn2 cores.

Sharding: data-parallel over batch (B=8) -- one batch element per NeuronCore.

v2 design (vs v1 baseline at 209us):
  * All matmuls 2-byte (fp16/bf16): every PE matmul runs 1 cycle/row,
    no fp32_mode=HIGH passes.
  * Bias algebra: softmax is invariant to per-query offsets, so the
    theta/phi biases reduce to a per-KEY term u[m] = (wp^T bt) . x[:,m].
    u is produced by a 129th column piggybacked on the g matmul and fed
    through the exp() bias operand -- all explicit bias adds vanish.
    (g's bias bg folds into the output bias since attn rows sum to 1.)
  * The per-query softmax normalizer commutes through the output matmul:
    wW @ (y/s) = (wW @ y)/s. Raw y is copied out of PSUM as bf16 right
    after the last chunk (freeing the y accumulator for the next quarter
    immediately); division by s happens after the wy matmul.
  * Flat software-pipelined loop over all 128 key-chunks (4 query
    quarters x 32): PE issues S(t) two iterations ahead of y(t-2) so PE
    never waits on ACT's exp; ACT runs exp() back-to-back at its
    ~1us/chunk floor with no gaps at quarter boundaries.
  * qkv (theta/phi/g) streamed per 512-col x-block, interleaved into
    quarter 0; x DMA interleaved block-major; theta blocks 4..7 deferred
    into quarters 1..2.
  * Engine placement: ACT = exp + phi copies; DVE = theta/g/u copies,
    3 of 4 sum-accumulator lanes, recip, y/o epilogues; Pool = the 4th
    accumulator lane; PE folds sums via ones-matmuls.

Softmax is computed without a per-row max: scores are ~N(0, 128) with
empirical |S| < ~91, so exp(S - 40) (a global shift -- softmax is
shift-invariant) stays comfortably inside fp32 range; raw wy values
stay below ~e^62, inside fp32/PSUM range.
"""

import numpy as np
from contextlib import ExitStack

import concourse.bass as bass
import concourse.mybir as mybir
import concourse.tile as tile
from concourse import bacc

P = 128          # partitions / inter channels
C = 256          # input channels
F32 = mybir.dt.float32
F16 = mybir.dt.float16
BF16 = mybir.dt.bfloat16
AF = mybir.ActivationFunctionType
ALU = mybir.AluOpType
CSHIFT = 40.0    # global score shift before exp (softmax-invariant)

N = 64 * 64      # 4096
NQ = 1024        # query-quarter width
MC = N // P      # 32 key chunks
NB = NQ // 512   # 512-col blocks per quarter
NQn = N // NQ    # 4 quarters

# accumulator lanes: chunk mc -> lane mc%4; lane 2 runs on Pool (GpSimd)
POOL_LANE = 2


def build_nc():
    """Single-core Bass module (SPMD: same NEFF on all 8 cores)."""
    nc = bacc.Bacc("TRN2", target_bir_lowering=False, debug=False)

    xh_d = nc.dram_tensor("xh", [P, 2 * N], F16, kind="ExternalInput").ap()
    wtT_d = nc.dram_tensor("wtT", [P, 2 * P], F16, kind="ExternalInput").ap()
    wpT_d = nc.dram_tensor("wpT", [P, 2 * P], F16, kind="ExternalInput").ap()
    wg_d = nc.dram_tensor("wg", [P, 2 * 129], F16, kind="ExternalInput").ap()
    wWT_d = nc.dram_tensor("wWT", [P, C], BF16, kind="ExternalInput").ap()
    bWp_d = nc.dram_tensor("bWp", [P, 2], F32, kind="ExternalInput").ap()
    out_d = nc.dram_tensor("out", [C, N], F32, kind="ExternalOutput").ap()

    xh_v = xh_d.rearrange("p (k n) -> p k n", k=2)
    out_v = out_d.rearrange("(k p) n -> k p n", p=P)

    with tile.TileContext(nc) as tc, ExitStack() as ctx:
        const = ctx.enter_context(tc.tile_pool(name="const", bufs=1))
        big = ctx.enter_context(tc.tile_pool(name="big", bufs=1))
        work = ctx.enter_context(tc.tile_pool(name="work", bufs=3))
        # PSUM: 8 banks of [128 x 2KB].
        #   rot_s: S-score tiles [128,1024]f32 x2 -> 4 banks
        #   psy:   y accumulator [128,1024]f32 x1 -> 2 banks
        #   misc:  th/ph/g/sum/wy [128,<=512] x2  -> 2 banks
        rot_s = ctx.enter_context(
            tc.tile_pool(name="rot_s", bufs=2, space="PSUM"))
        psy = ctx.enter_context(tc.tile_pool(name="psy", bufs=1, space="PSUM"))
        misc = ctx.enter_context(
            tc.tile_pool(name="misc", bufs=2, space="PSUM"))

        # ---- constants ----
        wtT_sb = const.tile([P, 2, P], F16, name="wtT_sb")
        wpT_sb = const.tile([P, 2, P], F16, name="wpT_sb")
        wg_sb = const.tile([P, 2, 129], F16, name="wg_sb")
        wWT_sb = const.tile([P, C], BF16, name="wWT_sb")
        bWp_sb = const.tile([P, 2], F32, name="bWp_sb")
        ones_sb = const.tile([P, P], BF16, name="ones_sb")
        nc.sync.dma_start(wtT_sb[:], wtT_d.rearrange("p (k i) -> p k i", k=2))
        nc.sync.dma_start(wpT_sb[:], wpT_d.rearrange("p (k i) -> p k i", k=2))
        nc.sync.dma_start(wg_sb[:], wg_d.rearrange("p (k i) -> p k i", k=2))
        nc.sync.dma_start(wWT_sb[:], wWT_d)
        nc.sync.dma_start(bWp_sb[:], bWp_d)
        nc.vector.memset(ones_sb[:], 1.0)

        # ---- x load: block-major, issued from the (idle) GpSimd queue so
        # the serial ~0.6us-per-DMA launch cost doesn't stack behind the
        # weight DMAs on the Sync queue ----
        xh_sb = big.tile([P, 2, N], F16, name="xh_sb")
        for blk in range(8):
            bsl = slice(blk * 512, (blk + 1) * 512)
            for k in range(2):
                nc.gpsimd.dma_start(xh_sb[:, k, bsl], xh_v[:, k, bsl])

        th_sb = big.tile([P, N], F16, name="th_sb")    # theta^T (i, n)
        ph_sb = big.tile([P, N], F16, name="ph_sb")    # phi (i, m)
        g_sb = big.tile([P, MC, P], BF16, name="g_sb")  # g0 (m_in, chunk, o)
        u_sb = big.tile([P, MC], F32, name="u_sb")     # per-key bias - 40

        # ---- emission helpers ----
        def emit_th(b):
            bsl = slice(b * 512, (b + 1) * 512)
            th_ps = misc.tile([P, 512], F32, tag="m", name="th_ps")
            nc.tensor.matmul(th_ps[:], wtT_sb[:, 0], xh_sb[:, 0, bsl],
                             start=True, stop=False)
            nc.tensor.matmul(th_ps[:], wtT_sb[:, 1], xh_sb[:, 1, bsl],
                             start=False, stop=True)
            nc.vector.tensor_copy(th_sb[:, bsl], th_ps[:])

        def emit_ph(b):
            bsl = slice(b * 512, (b + 1) * 512)
            ph_ps = misc.tile([P, 512], F32, tag="m", name="ph_ps")
            nc.tensor.matmul(ph_ps[:], wpT_sb[:, 0], xh_sb[:, 0, bsl],
                             start=True, stop=False)
            nc.tensor.matmul(ph_ps[:], wpT_sb[:, 1], xh_sb[:, 1, bsl],
                             start=False, stop=True)
            nc.scalar.copy(ph_sb[:, bsl], ph_ps[:])

        def emit_g(c):
            msl = slice(c * P, (c + 1) * P)
            g_ps = misc.tile([P, 129], F32, tag="m", name="g_ps")
            nc.tensor.matmul(g_ps[:], xh_sb[:, 0, msl], wg_sb[:, 0],
                             start=True, stop=False)
            nc.tensor.matmul(g_ps[:], xh_sb[:, 1, msl], wg_sb[:, 1],
                             start=False, stop=True)
            nc.vector.tensor_copy(g_sb[:, c], g_ps[:, 0:P])
            nc.vector.tensor_scalar_add(u_sb[:, c:c + 1], g_ps[:, P:P + 1],
                                        -CSHIFT)

        # per-t state carried across the flat loop
        s_tiles = {}    # t -> PSUM score tile
        exp_tiles = {}  # t -> SBUF exp tile
        acc = {}        # (q, lane) -> accumulator tile
        sum_half = {}   # (q, h) -> PSUM fold tile
        yps = {}        # q -> y accumulator PSUM tile
        ytraw = {}      # (q, b) -> unnormalized y, bf16 SBUF
        recips = {}     # (q, b) -> 1/sums tile

        def emit_S(t):
            q, mc = divmod(t, MC)
            msl = slice(mc * P, (mc + 1) * P)
            s_ps = rot_s.tile([P, NQ], F32, tag="s", name="s_ps")
            for b in range(NB):
                qb = slice(q * NQ + b * 512, q * NQ + (b + 1) * 512)
                nc.tensor.matmul(s_ps[:, b * 512:(b + 1) * 512],
                                 ph_sb[:, msl], th_sb[:, qb],
                                 start=True, stop=True)
            s_tiles[t] = s_ps

        def emit_exp(t):
            q, mc = divmod(t, MC)
            e = work.tile([P, NQ], BF16, tag="exp", bufs=8, name="exp_sb")
            nc.scalar.activation(e[:], s_tiles.pop(t)[:], AF.Exp,
                                 bias=u_sb[:, mc:mc + 1])
            exp_tiles[t] = e

        def emit_y(t):
            q, mc = divmod(t, MC)
            if mc == 0:
                yps[q] = psy.tile([P, NQ], F32, tag="y", name="y_ps")
            e = exp_tiles[t]
            for b in range(NB):
                bsl = slice(b * 512, (b + 1) * 512)
                nc.tensor.matmul(yps[q][:, bsl], g_sb[:, mc], e[:, bsl],
                                 start=(mc == 0), stop=(mc == MC - 1),
                                 skip_group_check=True)

        def lane_of(q, mc):
            # (lane, is_init, on_pool). Final quarter uses a custom map so
            # lanes finish (and fold) early, shortening the tail chain.
            if q == NQn - 1:
                if mc <= 9:
                    return 0, mc == 0, False
                if mc <= 19:
                    return 1, mc == 10, False
                if mc in (20, 22, 24, 26):
                    return 2, mc == 20, mc != 20
                return 3, mc == 21, False
            j = mc % 4
            # Pool's ~2us adds must finish well before the fold matmuls
            return j, mc < 4, j == POOL_LANE and 4 <= mc <= 22

        def emit_acc(t):
            q, mc = divmod(t, MC)
            e = exp_tiles.pop(t)
            j, init, pool = lane_of(q, mc)
            if init:
                a = work.tile([P, NQ], BF16, tag=f"acc{j}", bufs=2,
                              name=f"acc{j}_sb")
                nc.vector.tensor_copy(a[:], e[:])  # init always on DVE (fast)
                acc[(q, j)] = a
            else:
                a = acc[(q, j)]
                eng = nc.gpsimd if pool else nc.vector
                eng.tensor_add(a[:], a[:], e[:])

        def emit_ytraw(q):
            for b in range(NB):
                yt = work.tile([P, 512], BF16, tag="yt", bufs=2, name="yt_sb")
                nc.vector.tensor_copy(yt[:], yps[q][:, b * 512:(b + 1) * 512])
                ytraw[(q, b)] = yt
            del yps[q]

        def emit_fold(q, j, start, stop):
            a = acc.pop((q, j))
            for h in range(2):
                if (q, h) not in sum_half:
                    sum_half[(q, h)] = misc.tile([P, 512], F32, tag="m",
                                                 name="sum_ps")
                nc.tensor.matmul(
                    sum_half[(q, h)][:], ones_sb[:],
                    a[:, h * 512:(h + 1) * 512],
                    start=start, stop=stop, skip_group_check=True)

        def emit_recip(q):
            for b in range(NB):
                r = work.tile([P, 512], F32, tag="recip", bufs=2,
                              name="recip_sb")
                nc.vector.reciprocal_approx_fast(r[:], sum_half.pop((q, b))[:])
                # normalize before the wy matmul (wW@(y/s) == (wW@y)/s):
                # halves the number of per-output normalize ops
                yn = work.tile([P, 512], BF16, tag="yn", bufs=2, name="yn_sb")
                nc.vector.tensor_mul(yn[:], ytraw.pop((q, b))[:], r[:])
                recips[(q, b)] = yn

        def emit_wy(q, b):
            # output block b (512 queries), both channel halves
            qb = slice(q * NQ + b * 512, q * NQ + (b + 1) * 512)
            for h in range(2):
                wy_ps = misc.tile([P, 512], F32, tag="m", name="wy_ps")
                nc.tensor.matmul(wy_ps[:], wWT_sb[:, h * P:(h + 1) * P],
                                 recips[(q, b)][:], start=True, stop=True)
                o = work.tile([P, 512], F32, tag="o", bufs=4, name="o_sb")
                nc.vector.scalar_tensor_tensor(
                    o[:], wy_ps[:], bWp_sb[:, h:h + 1], xh_sb[:, h, qb],
                    op0=ALU.add, op1=ALU.add)
                # tail outputs go out on the (by then idle) ACT queue so the
                # Sync queue's serial ~0.6us DMA launches don't stack up
                eng = nc.scalar if q == NQn - 1 else nc.sync
                eng.dma_start(out_v[h, :, qb], o[:])

        # epilogue schedules (rel offset from quarter start -> pieces).
        # Interior quarters: spread thinly after the quarter ends. Final
        # quarter: lanes fold as soon as they finish so only lane 3's fold
        # sits on the tail chain.
        EPI_INNER = {
            32: [('ytraw',), ('fold', POOL_LANE, True, False)],
            33: [('fold', 0, False, False)],
            35: [('fold', 1, False, False)],
            37: [('fold', 3, False, True)],
            38: [('recip',)],
            39: [('wy', 0)],
            40: [('wy', 1)],
        }
        EPI_LAST = {
            12: [('fold', 0, True, False)],
            22: [('fold', 1, False, False)],
            29: [('fold', 2, False, False)],
            32: [('ytraw',)],
            33: [('fold', 3, False, True)],
            34: [('recip',)],
            35: [('wy', 0)],
            36: [('wy', 1)],
        }

        def emit_epilogue_piece(qe, piece):
            if piece[0] == 'ytraw':
                emit_ytraw(qe)
            elif piece[0] == 'fold':
                emit_fold(qe, piece[1], start=piece[2], stop=piece[3])
            elif piece[0] == 'recip':
                emit_recip(qe)
            elif piece[0] == 'wy':
                emit_wy(qe, piece[1])

        # ---- PE warmup: ~3.4us of dummy matmuls while the first x blocks
        # are still in flight, so the PE pstate governor reaches full clock
        # before real work starts ----
        warm_sb = const.tile([P, 512], BF16, name="warm_sb")
        nc.vector.memset(warm_sb[:], 0.0)
        warm_ps = rot_s.tile([P, NQ], F32, tag="s", name="warm_ps")
        for _ in range(8):
            nc.tensor.matmul(warm_ps[:, 0:512], ones_sb[:], warm_sb[:],
                             start=True, stop=True, skip_group_check=True)

        # ---- prologue: minimum to start S(0)/exp(0) ----
        emit_th(0)
        emit_ph(0)
        emit_th(1)
        emit_g(0)

        # theta block b feeds quarter b//2; blocks 2,3 must land in Q0,
        # 4..7 are deferred into Q1/Q2 to unload quarter 0.
        TH_AT = {11: 2, 19: 3, 36: 4, 44: 5, 68: 6, 76: 7}
        # remaining phi/g units (b=2..7) stream through quarter 0, split
        # finely (ph / 2 g / 2 g on consecutive iterations) so the PE's
        # S(t) stream is never displaced by a burst
        PH_AT = {4 * (b - 1): b for b in range(2, 8)}
        G_AT = {1: (1, 2, 3), 2: (4, 5), 3: (6, 7)}  # early g chunks
        for b in range(2, 8):
            G_AT[4 * (b - 1) + 1] = (4 * b, 4 * b + 1)
            G_AT[4 * (b - 1) + 2] = (4 * b + 2, 4 * b + 3)
        PH_EARLY = {2: 1}                          # ph block 1 at t=2

        # ---- flat pipelined main loop ----
        T = NQn * MC  # 128
        for t in range(T + 9):
            # exp first: its bias operand makes it depend (coarsely) on all
            # u_sb writes emitted before it, so the qkv-unit injections for
            # this iteration must come after it
            if 0 <= t - 1 < T:
                emit_exp(t - 1)
            if t < T:
                if t in PH_EARLY:
                    emit_ph(PH_EARLY[t])
                if t in G_AT:
                    for c in G_AT[t]:
                        emit_g(c)
                if t in PH_AT:
                    emit_ph(PH_AT[t])
                if t in TH_AT:
                    emit_th(TH_AT[t])
                emit_S(t)
            ty = t - 2
            if ty >= 0:
                for qe in range(NQn):
                    rel = ty - MC * qe
                    sched = EPI_LAST if qe == NQn - 1 else EPI_INNER
                    for piece in sched.get(rel, ()):
                        emit_epilogue_piece(qe, piece)
                if ty < T:
                    emit_y(ty)
                    emit_acc(ty)

    nc.compile()
    return nc


_CACHE = {}


def _built():
    if "nc" not in _CACHE:
        _CACHE["nc"] = build_nc()
    return _CACHE["nc"]


def make_in_maps(x, wg, bg, wt, bt, wp, bp, wW, bW):
    """Host-side prep: per-core input dicts (core b <- batch b)."""
    x = np.asarray(x, np.float32)
    B, C_, H, W = x.shape
    xf = x.reshape(B, C_, H * W)
    wg, bg, wt, bt, wp, bp, wW, bW = [
        np.asarray(a, np.float32) for a in (wg, bg, wt, bt, wp, bp, wW, bW)]

    def pack(w):  # (128, 256) conv weight -> [part, k, i] fp16 lhsT chunks
        return np.ascontiguousarray(
            w.T.reshape(2, P, P).transpose(1, 0, 2).reshape(P, 2 * P)
        ).astype(np.float16)

    # g matmul rhs augmented with the per-key bias column:
    #   u[m] = sum_c (wp^T bt)[c] x[c, m]
    w_u = (wp.T @ bt).astype(np.float32)              # (256,)
    wg_aug = np.concatenate(
        [wg.T.reshape(2, P, P), w_u.reshape(2, P, 1)], axis=2)  # (2,128,129)
    wg_aug = np.ascontiguousarray(
        wg_aug.transpose(1, 0, 2).reshape(P, 2 * 129)).astype(np.float16)

    bWp = (wW @ bg + bW).astype(np.float32)           # fold bg into bW
    bWp = np.ascontiguousarray(bWp.reshape(2, P).T)   # (128, 2)

    from ml_dtypes import bfloat16
    shared = {
        "wtT": pack(wt), "wpT": pack(wp), "wg": wg_aug,
        "wWT": np.ascontiguousarray(wW.T).astype(bfloat16),
        "bWp": bWp,
    }
    in_maps = []
    for b in range(B):
        xh = np.ascontiguousarray(
            xf[b].reshape(2, P, H * W).transpose(1, 0, 2).reshape(P, 2 * H * W)
        ).astype(np.float16)
        in_maps.append({"xh": xh, **shared})
    return in_maps


def kernel(x, wg, bg, wt, bt, wp, bp, wW, bW):
    from concourse.bass_utils import run_bass_kernel_spmd

    B, C_, H, W = np.asarray(x).shape
    in_maps = make_in_maps(x, wg, bg, wt, bt, wp, bp, wW, bW)
    nc = _built()
    res = run_bass_kernel_spmd(nc, in_maps, core_ids=list(range(B)))
    out = np.stack([res.results[b]["out"] for b in range(B)])
    return out.reshape(B, C_, H, W).astype(np.float32)
